# revision 12
# baseline (speedup 1.0000x reference)
"""FP8 GEMM kernel (MixLinear) for 8 trn2 NeuronCores.

Reference computation:
    s      = max(|x|) / 448                        (global fp32 scalar)
    q_x    = e4m3fn(clip(x / s, +-448))            (OCP e4m3fn)
    q_w    = e4m3fn(clip(w, +-448))                (scale_weight = 1)
    y      = (q_x @ q_w.T) * s + bias              (fp32 accum -> fp16)

Strategy: data-parallel over the 16384 token rows (2048 rows per core).
Host does layout only (transpose so the contraction dim d_in lands on
SBUF partitions, slice, and the static scale-1 weight cast to fp8 --
init-time work in the reference semantics).  Device does amax, a
cross-core max exchange, quantization, DoubleRow fp8 matmul and
scale+bias eviction.

TRN e4m3 tops out at 240 (vs OCP 448), so x is quantized at half scale:
    q_half = trn_e4m3(x * (224/gmax))  ==  ocp_e4m3(x / s) / 2
exactly for all magnitudes >= 2^-6 * s (below that the two grids differ
by one subnormal bit -- negligible).  Weights (|w| <= 1/sqrt(2048)) are
in the range where the TRN and OCP grids agree exactly, so they are
quantized at scale 1.  The output scale is then 2*s = gmax/224.

Phase 1 (the critical prefix -- the global max gates quantization):
  - x streams in as 16 half-tile DMAs on the sync HWDGE ring with the
    fp8 weights queued behind it (x gets all the HBM bandwidth first).
  - amax is ONE DVE abs_max-accumulate per half tile (fp16, 2x tier,
    1.1us per 0.5MB -- faster than the 1.5us DMA pace), folded at the
    end by a tensor_tensor_reduce.
  - cross-core max: 7 single-peer XOR-relative remote-DMA broadcasts,
    desc-gen'd during the x load, fired by one trigger_dma when the
    local max is ready.  Sender c's slot-k send lands in column k of
    receiver c^k's gather tile: every column has exactly one writer,
    identical SPMD program, no barrier, ~3us vs ~28us for the gpsimd
    collective AllGather.
  - dummy fp16 matmuls paced by tile arrivals keep the PE clock-gate
    warm through phase 1.

Phase 2: weights are the stationary matmul operand, tokens stream, so
PSUM comes out as [d_out, tokens] and both the output scale and the
bias are per-partition: evictions split across DVE (tensor_scalar
mult+add) and ACT (activation scale+bias).  y is produced transposed;
the host transposes it back.
"""

import numpy as np

B, S, D_IN, D_OUT = 2, 8192, 2048, 2048
N_CORES = 8
TOK = B * S                  # 16384
TOK_PC = TOK // N_CORES      # 2048 token rows per core
P = 128
KP = D_IN // (2 * P)         # 8 k-pairs of 256 (DoubleRow granularity)
TC = 512                     # moving-operand token chunk
NTC = TOK_PC // TC           # 4 token chunks
NOUT = D_OUT // P            # 16 output column tiles of 128
NQ = 4                       # psum tiles per group

EXCHANGE = "rdma"            # "rdma" | "cc"

_compiled = None


def _build():
    import concourse.bacc as bacc
    import concourse.tile as tile
    from concourse import mybir
    from concourse.masks import make_identity

    f16 = mybir.dt.float16
    f32 = mybir.dt.float32
    f8 = mybir.dt.float8e4
    Alu = mybir.AluOpType
    Axis = mybir.AxisListType
    Act = mybir.ActivationFunctionType

    nc = bacc.Bacc("TRN2", target_bir_lowering=False, debug=False,
                   num_devices=N_CORES)

    # xt: x^T shard [d_in, tok_pc]; wt8: w^T [d_in, d_out] fp8 (replicated)
    xt = nc.dram_tensor("xt", [D_IN, TOK_PC], f16, kind="ExternalInput")
    wt8 = nc.dram_tensor("wt8", [D_IN, D_OUT], f8, kind="ExternalInput")
    bias = nc.dram_tensor("bias", [D_OUT], f16, kind="ExternalInput")
    # y^T [d_out, tok_pc]; host transposes back
    yt = nc.dram_tensor("yt", [D_OUT, TOK_PC], f16, kind="ExternalOutput")

    if EXCHANGE == "cc":
        cc_in = nc.dram_tensor("cc_in", [16], f32)
        cc_out = nc.dram_tensor("cc_out", [16 * N_CORES], f32,
                                addr_space="Shared")
        groups = [list(range(N_CORES))]
    else:
        rsem = nc.alloc_semaphore("gmax_rsem")
        lsem = nc.alloc_semaphore("gmax_lsem")

    with tile.TileContext(nc) as tc:
        with (
            tc.tile_pool(name="xpool", bufs=KP) as xpool,
            tc.tile_pool(name="qxpool", bufs=KP) as qxpool,
            tc.tile_pool(name="qwpool", bufs=KP) as qwpool,
            tc.tile_pool(name="small", bufs=1) as small,
            tc.tile_pool(name="ypool", bufs=6) as ypool,
            tc.tile_pool(name="psum", bufs=8, space="PSUM") as psum,
        ):
            # gather tile for the cross-core max exchange: column k is
            # written remotely by core (self XOR k).  memset FIRST so a
            # fast peer's early write can't be clobbered later.
            gather = small.tile([P, N_CORES], f32)
            nc.vector.memset(gather[:], 0.0)

            # identity for the PE-transpose partition fold
            ident = small.tile([P, P], f32)
            make_identity(nc, ident[:])

            # per-half-tile |max| partials: first NGP halves reduce fully on
            # GPSIMD (axis XYZWC -> scalar), the rest per-partition on DVE
            NGP = 4
            gp_row = small.tile([1, NGP], f32)
            pmax = small.tile([P, 2 * KP - NGP], f32)
            lmax = small.tile([P, 1], f32)

            if EXCHANGE == "rdma":
                # desc-gen for the 7 peer sends now (gpsimd is idle; the
                # transfers only fire at trigger_dma, which Tile orders
                # after lmax's producer via the deferred source read).
                for k in range(1, N_CORES):
                    nc.gpsimd.remote_dma_broadcast(
                        out_ap=gather[:, k:k + 1],
                        in_ap=lmax[:, 0:1],
                        remote_sem=rsem,
                        local_sem=lsem,
                        rdests=[(0, k) if i == k else None
                                for i in range(N_CORES)],
                    )

            # ---- Phase A: load x^T shard; abs-max as half-tiles arrive ----
            x_sb = []
            for j in range(KP):
                t = xpool.tile([P, 2, TOK_PC], f16, tag="xsb")
                src = xt[2 * j * P:(2 * j + 2) * P, :]
                src = src.rearrange("(p t) m -> p t m", t=2)
                for h in range(2):
                    nc.sync.dma_start(t[:, h, :], src[:, h, :])
                    hh = 2 * j + h
                    if hh < NGP:
                        nc.gpsimd.tensor_reduce(
                            out=gp_row[:, hh:hh + 1], in_=t[:, h, :],
                            axis=Axis.XYZWC, op=Alu.max,
                            apply_absolute_value=True)
                    else:
                        nc.vector.tensor_reduce(
                            out=pmax[:, hh - NGP:hh - NGP + 1], in_=t[:, h, :],
                            axis=Axis.X, op=Alu.max,
                            apply_absolute_value=True)
                x_sb.append(t)
                # dummy fp16 matmul paced by this tile's arrival keeps the
                # PE clock-gate warm through phase 1 (scratch psum)
                warm = psum.tile([P, TC], f32, tag="ps", name=f"warm{j}")
                nc.tensor.matmul(warm[:], t[:, 0, 0:P], t[:, 0, 0:TC],
                                 start=True, stop=True)

            # fp8 weights queued behind x on the same HWDGE ring: strict
            # FIFO gives x all the HBM bandwidth first.
            qw = []
            for j in range(KP):
                qt = qwpool.tile([P, 2, D_OUT], f8, tag="qw")
                src = wt8[2 * j * P:(2 * j + 2) * P, :]
                nc.sync.dma_start(qt[:], src.rearrange("(p t) n -> p t n", t=2))
                qw.append(qt)

            # bias on the scalar (ACT) ring as [128, 16]: column n = bias
            # slice for output tile n -> per-partition bias operand
            bias_f16 = small.tile([P, NOUT], f16)
            nc.scalar.dma_start(bias_f16[:],
                                bias.rearrange("(n p) -> p n", p=P))
            bias_sb = small.tile([P, NOUT], f32)
            nc.vector.tensor_copy(out=bias_sb[:], in_=bias_f16[:])

            # fold: per-partition max of the DVE partials, then merge the
            # gpsimd scalars into partition 0 (partition fold comes after
            # the exchange, via PE transpose)
            nc.vector.tensor_reduce(out=lmax[:], in_=pmax[:], axis=Axis.X,
                                    op=Alu.max)
            gpm = small.tile([1, 1], f32)
            nc.vector.tensor_reduce(out=gpm[:], in_=gp_row[:], axis=Axis.X,
                                    op=Alu.max)
            nc.vector.tensor_tensor(out=lmax[0:1, :], in0=lmax[0:1, :],
                                    in1=gpm[:], op=Alu.max)

            # ---- Phase B: cross-core max of lmax ----
            if EXCHANGE == "rdma":
                nc.vector.tensor_copy(out=gather[:, 0:1], in_=lmax[:])
                nc.gpsimd.trigger_dma(count=None)
                gvec = small.tile([P, 1], f32)
                # the wait on rsem (7 senders x 2 increments) is attached
                # POST-scheduling: the tile-schedule sim is single-core and
                # would deadlock on a semaphore only remote cores increment.
                gather_red = nc.vector.tensor_reduce(
                    out=gvec[:], in_=gather[:], axis=Axis.X, op=Alu.max)
            else:
                lmax16 = small.tile([1, 16], f32)
                gvec = lmax  # partition fold below handles the rest
            # fold partitions via PE transpose
            gvec_t = psum.tile([1, P], f32, tag="ps", name="gvt")
            nc.tensor.transpose(gvec_t[:], gvec[:], ident[:])
            gmax0 = small.tile([1, 1], f32)
            if EXCHANGE == "cc":
                nc.vector.memset(lmax16[:], 0.0)
                nc.vector.tensor_reduce(out=lmax16[:, 0:1], in_=gvec_t[:],
                                        axis=Axis.X, op=Alu.max)
                nc.sync.dma_start(cc_in[:], lmax16[:])
                nc.gpsimd.collective_compute(
                    "AllGather", Alu.bypass, replica_groups=groups,
                    ins=[cc_in.ap().opt()], outs=[cc_out.ap().opt()])
                gall = small.tile([1, 16 * N_CORES], f32)
                nc.sync.dma_start(gall[:], cc_out[None, :])
                nc.vector.tensor_reduce(out=gmax0[:], in_=gall[:],
                                        axis=Axis.X, op=Alu.max)
            else:
                nc.vector.tensor_reduce(out=gmax0[:], in_=gvec_t[:],
                                        axis=Axis.X, op=Alu.max)

            # scale math on partition 0: col0 = inv_half, col1 = out_scale
            sc = small.tile([1, 2], f32)
            nc.vector.reciprocal(sc[:, 0:1], gmax0[:])
            nc.vector.tensor_scalar_mul(sc[:, 0:1], sc[:, 0:1], 224.0)
            nc.vector.tensor_scalar_mul(sc[:, 1:2], gmax0[:], 1.0 / 224.0)
            scales = small.tile([P, 2], f32)
            nc.gpsimd.partition_broadcast(scales[:], sc[:], P)
            inv_half = scales[:, 0:1]
            out_scale = scales[:, 1:2]

            # ---- Phase C: quantize x at half scale, token-chunk order ----
            # token chunk 0 of all k-pairs first so the matmul stream
            # (token-chunk major, j-minor) starts immediately; j order
            # matches the matmul's j consumption order.  DVE/ACT split.
            qx = []
            for j in range(KP):
                qx.append(qxpool.tile([P, 2, TOK_PC], f8, tag="qx",
                                      name=f"qx{j}"))
            for t in range(NTC):
                lo, hi = t * TC, (t + 1) * TC
                for j in range(KP):
                    if j % 8 < 5:
                        nc.vector.tensor_scalar(out=qx[j][:, :, lo:hi],
                                                in0=x_sb[j][:, :, lo:hi],
                                                scalar1=inv_half[:, 0:1],
                                                scalar2=None, op0=Alu.mult)
                    else:
                        nc.scalar.activation(qx[j][:, :, lo:hi],
                                             x_sb[j][:, :, lo:hi],
                                             Act.Copy, scale=inv_half[:, 0:1])

            # ---- Phase D: DoubleRow fp8 matmul + fused scale/bias ----
            # stationary = weight tile [128k, 2, 128 dout]; moving = token
            # chunk [128k, 2, 512 tok]; psum = [128 dout, 512 tok]
            for t in range(NTC):
                lo, hi = t * TC, (t + 1) * TC
                for q in range(NOUT // NQ):
                    ps = [psum.tile([P, TC], f32, tag="ps", name=f"ps{n}")
                          for n in range(NQ)]
                    for j in range(KP):
                        rhs = qx[j][:, :, lo:hi]
                        for n in range(NQ):
                            no = q * NQ + n
                            nc.tensor.matmul(
                                ps[n][:],
                                qw[j][:, :, no * P:(no + 1) * P],
                                rhs,
                                start=(j == 0), stop=(j == KP - 1),
                                perf_mode=mybir.MatmulPerfMode.DoubleRow)
                    for n in range(NQ):
                        no = q * NQ + n
                        ysb = ypool.tile([P, TC], f16, tag="ysb")
                        if n % 2 == 0:
                            nc.vector.tensor_scalar(
                                out=ysb[:], in0=ps[n][:],
                                scalar1=out_scale[:, 0:1],
                                scalar2=bias_sb[:, no:no + 1],
                                op0=Alu.mult, op1=Alu.add)
                        else:
                            nc.scalar.activation(
                                ysb[:], ps[n][:], Act.Identity,
                                scale=out_scale[:, 0:1],
                                bias=bias_sb[:, no:no + 1])
                        nc.scalar.dma_start(
                            yt[no * P:(no + 1) * P, lo:hi], ysb[:])

    if EXCHANGE == "rdma":
        # HW-only wait for the 7 remote deliveries (2 sem incs each),
        # spliced in POST-scheduling right before the gather reduce: the
        # tile scheduler's single-core simulator would deadlock on a
        # semaphore only remote cores increment.
        w = nc.vector.wait_ge(rsem, 2 * (N_CORES - 1))
        fn = nc.m.functions[0]
        for blk in fn.blocks:
            if w.ins in blk.instructions:
                blk.instructions.remove(w.ins)
                break
        for blk in fn.blocks:
            insts = blk.instructions
            if gather_red.ins in insts:
                insts.insert(insts.index(gather_red.ins), w.ins)
                break
        else:
            raise RuntimeError("gather reduce not found in any block")

    nc.compile()
    return nc


def _get_compiled():
    global _compiled
    if _compiled is None:
        _compiled = _build()
    return _compiled


def _quant_w_host(weight):
    """Static scale-1 e4m3fn cast of the weight on the host (init-time
    work in the reference model).  |w| <= 1/45 so the OCP and TRN grids
    agree bit-for-bit."""
    import ml_dtypes
    return weight.astype(np.float32).astype(ml_dtypes.float8_e4m3fn)


def run(x, weight, bias, **kw):
    """Shard + run on 8 cores; returns (full_output, BassKernelResults)."""
    from concourse.bass_utils import run_bass_kernel_spmd

    nc = _get_compiled()

    x = np.asarray(x, dtype=np.float16)
    weight = np.asarray(weight, dtype=np.float16)
    bias = np.asarray(bias, dtype=np.float16)
    xt = np.ascontiguousarray(x.reshape(TOK, D_IN).T)          # [d_in, tok]
    wt8 = np.ascontiguousarray(_quant_w_host(weight).T)        # [d_in, d_out]
    in_maps = []
    for i in range(N_CORES):
        in_maps.append({
            "xt": np.ascontiguousarray(xt[:, i * TOK_PC:(i + 1) * TOK_PC]),
            "wt8": wt8,
            "bias": bias,
        })
    res = run_bass_kernel_spmd(nc, in_maps, core_ids=list(range(N_CORES)), **kw)
    # yt is [d_out, tok_pc] per core: transpose back and concat over tokens
    out = np.concatenate(
        [np.ascontiguousarray(res.results[i]["yt"].T) for i in range(N_CORES)],
        axis=0)
    return out.reshape(B, S, D_OUT), res


def kernel(x, weight, bias):
    out, _ = run(x, weight, bias)
    return out
